# revision 50
# baseline (speedup 1.0000x reference)
"""Trainium2 Bass kernel for nn_EnvironmentEmbedder.

Sharding: pure data parallel. Core i processes batch slice [128*i : 128*(i+1)],
with batch elements mapped to SBUF partitions ([128, free] tiles everywhere).

Per-core compute layout (output = [128, 161*625] bf16, channel-major free dim;
the host widens to f32 after gather — tolerance is 2e-2 and the worst output
channel carries <= 3 RNE bf16 roundings ~= 1.2e-2):
  ch   0..127  (static_c + dynamic_c) * obs      streamed in 4-channel chunks,
                                                 f32 add+mul on DVE (3 of 4)
                                                 or GpSimd (every 4th)
  ch 128..130  obstacle/ocur/obs * obs
  ch 131..136  shuffle(prev_visitations)_j * 0.5 * obs
  ch 137       sum_k(vis_k) * obs
  ch 138..139  leader/follower * obs
  ch 140..145  shuffle(all_prev_targets)_j * 0.5 * obs
  ch 146..151  shuffle(previous_target)_j * obs
  ch 152       0.5 * sum_k(atgt_k) * obs
  ch 153       sum_k(ptgt_k) * obs
  ch 154       1.0
  ch 155..160  one_hot(rot)
where obs := observability_in_memory.

The egocentric shuffle out_j = x_{(j - rot) % 6} is computed with per-partition
one-hot masks R_r = (rot == r):  out_j = sum_r R_r * x_{(j-r)%6}.  Because the
masks select exactly one term per partition, a bf16 accumulator stays exact for
the +0 steps, so the whole small-channel pipeline runs in bf16.

All small inputs ride one host-packed bf16 tensor ("small_pack"); [obs, rot]
lead it and are fetched by a separate tiny DMA so the env stream's obs
dependency clears ~2us in instead of after the full pack transfer. The two env
tensors are host-interleaved chunk-wise ([s0 d0 s1 d1 ...]) so each env chunk
is ONE 2.56 MB load DMA: the HWDGE semaphore pool covers only ~10 in-flight
DMAs per ring, and fewer/bigger DMAs keep the ring from draining dry while a
kick waits on semaphore recycling.

DMA queues: all HBM->SBUF loads ride the SP HWDGE ring; all SBUF->HBM stores
ride the Activation HWDGE ring, kicked one chunk late so the ACT engine never
sits in a store's wait-on-compute. f32 tensor_tensor is port-limited to
1 elem/cycle on DVE, so every 4th env chunk runs on the otherwise-idle GpSimd
(own tile pools — a GpSimd chunk takes ~2x a DVE chunk and must not stall DVE
chunk load-tile recycling).
"""

import sys

sys.path.insert(0, "/opt/trn_rl_repo")

from contextlib import ExitStack

import ml_dtypes
import numpy as np

import concourse.bass as bass
import concourse.tile as tile
from concourse import bacc, mybir
from concourse.bass_utils import run_bass_kernel_spmd

F32 = mybir.dt.float32
BF16 = mybir.dt.bfloat16
ALU = mybir.AluOpType
NP_BF16 = ml_dtypes.bfloat16

B = 1024
N_CORES = 8
BS = B // N_CORES  # 128 batch elements per core = SBUF partitions
EMB = 128
HW = 625  # 25*25
NROT = 6
NCH = EMB + 33  # 161 output channels

ENV_CHUNK = 4  # env channels per streamed tile
PACK_LAYOUT = [("obs", HW), ("rot", 1), ("obstacle", HW), ("ocur", HW),
               ("leader", HW), ("follower", HW), ("vis", NROT * HW),
               ("atgt", NROT * HW), ("ptgt", NROT * HW)]
PACK_W = sum(w for _, w in PACK_LAYOUT)  # 14376 bf16 per partition
HEAD_W = HW + 1  # obs + rot: fetched first
STAGE_CHUNKS = [(c, 3) for c in range(128, 161, 3)]  # 11 groups of 3 channels


def build_body(nc, tc, ctx, t_in, t_out):
    pool = ctx.enter_context(tc.tile_pool(name="resident", bufs=1))
    stage_b_pool = ctx.enter_context(tc.tile_pool(name="stage_b", bufs=2))
    env_s_pool = ctx.enter_context(tc.tile_pool(name="env_s", bufs=5))
    env_o_pool = ctx.enter_context(tc.tile_pool(name="env_o", bufs=3))
    env_sg_pool = ctx.enter_context(tc.tile_pool(name="env_sg", bufs=2))
    env_og_pool = ctx.enter_context(tc.tile_pool(name="env_og", bufs=2))

    # ---- resident load: small tensors host-packed in bf16, obs+rot first;
    # the pack REST is kicked later (inside the env loop at c==2) so the
    # first env chunk transfers lead the ring ----
    pack_t = pool.tile([BS, PACK_W], BF16, tag="pack")
    nc.sync.dma_start(pack_t[:, :HEAD_W], t_in["small_pack"][:, :HEAD_W])
    cols = {}
    off = 0
    for name, wdt in PACK_LAYOUT:
        cols[name] = pack_t[:, off:off + wdt]
        off += wdt
    obs_t = cols["obs"]
    rot_t = cols["rot"]
    vis_t = cols["vis"]
    atgt_t = cols["atgt"]
    ptgt_t = cols["ptgt"]

    # f32 obs replica for the env stream (bf16 source -> one 0.39% rounding)
    obs_f = pool.tile([BS, HW], F32, tag="obs_f")
    nc.vector.tensor_copy(obs_f[:], obs_t)
    scratch = pool.tile([BS, HW], F32, tag="scratch")  # f32 accum for ch sums

    def bc(ap, n):  # broadcast [BS, HW] along a virtual channel axis
        return ap.rearrange("p (o x) -> p o x", o=1).to_broadcast([BS, n, HW])

    def by_ch(ap, n):
        return ap.rearrange("p (c x) -> p c x", c=n)

    # ---- constants: masks ----
    R = []   # R[r]  = (rot == r)            [128, 1] f32
    Rh = []  # Rh[r] = 0.5 * (rot == r)
    for r in range(NROT):
        rt = pool.tile([BS, 1], F32, tag=f"R{r}")
        nc.vector.tensor_scalar(rt[:], rot_t, float(r), None, op0=ALU.is_equal)
        R.append(rt)
        rh = pool.tile([BS, 1], F32, tag=f"Rh{r}")
        nc.vector.tensor_scalar_mul(rh[:], rt[:], 0.5)
        Rh.append(rh)

    # premultiply the 6-channel tensors by obs (in place, all-bf16); emitted
    # from the env loop at c==3 — DVE is in-order, so emitting these before
    # chunk 0 would stall the first env adds behind the pack-rest DMA
    def emit_premults():
        for xt in (vis_t, atgt_t, ptgt_t):
            nc.vector.tensor_mul(by_ch(xt, NROT), by_ch(xt, NROT),
                                 bc(obs_t, NROT))

    def emit_shuffle(slot, xp, masks, j):
        # slot = sum_r masks[r] * xp[:, ((j - r) % 6)]
        nc.scalar.mul(slot, xp[:, j * HW:(j + 1) * HW], masks[0][:])
        for r in range(1, NROT):
            k = (j - r) % NROT
            nc.vector.scalar_tensor_tensor(
                slot, xp[:, k * HW:(k + 1) * HW], masks[r][:], slot,
                op0=ALU.mult, op1=ALU.add)

    def emit_chsum(slot, xp, scale):
        # f32 scratch accumulation (bf16 terms), one rounding at the cast;
        # on GpSimd to keep DVE clear for the env stream
        nc.gpsimd.tensor_add(scratch[:], xp[:, 0:HW], xp[:, HW:2 * HW])
        for k in range(2, NROT):
            nc.gpsimd.tensor_add(scratch[:], scratch[:],
                                 xp[:, k * HW:(k + 1) * HW])
        nc.scalar.mul(slot, scratch[:], scale)

    def emit_channel(ch, slot):
        if ch == 128:
            nc.gpsimd.tensor_mul(slot, cols["obstacle"], obs_t)
        elif ch == 129:
            nc.gpsimd.tensor_mul(slot, cols["ocur"], obs_t)
        elif ch == 130:
            nc.gpsimd.tensor_mul(slot, obs_t, obs_t)
        elif 131 <= ch <= 136:
            emit_shuffle(slot, vis_t, Rh, ch - 131)
        elif ch == 137:
            emit_chsum(slot, vis_t, 1.0)
        elif ch == 138:
            nc.gpsimd.tensor_mul(slot, cols["leader"], obs_t)
        elif ch == 139:
            nc.gpsimd.tensor_mul(slot, cols["follower"], obs_t)
        elif 140 <= ch <= 145:
            emit_shuffle(slot, atgt_t, Rh, ch - 140)
        elif 146 <= ch <= 151:
            emit_shuffle(slot, ptgt_t, R, ch - 146)
        elif ch == 152:
            emit_chsum(slot, atgt_t, 0.5)
        elif ch == 153:
            emit_chsum(slot, ptgt_t, 1.0)
        elif ch == 154:
            nc.gpsimd.memset(slot, 1.0)
        else:  # 155..160: compass one-hot = Identity(0*obs + R[r])
            nc.scalar.activation(
                slot, obs_t, mybir.ActivationFunctionType.Identity,
                bias=R[ch - 155][:], scale=0.0)

    # ---- env stream interleaved with the small channels ----
    ch_queue = []
    for ck, (start_ch, n_ch) in enumerate(STAGE_CHUNKS):
        for i in range(n_ch):
            ch_queue.append((ck, start_ch, n_ch, i))
    stage_tiles = {}

    # all stores are kicked one chunk late: by then their compute is done, so
    # the ACT ring (which also runs shuffle first-ops and casts) never sits
    # in a kick's wait-on-compute
    pending_store = []

    def emit_small(budget):
        while budget > 0 and ch_queue:
            ck, start_ch, n_ch, i = ch_queue.pop(0)
            if ck not in stage_tiles:
                stage_tiles[ck] = stage_b_pool.tile(
                    [BS, n_ch * HW], BF16, tag="stage", name=f"stage{ck}")
            emit_channel(start_ch + i, stage_tiles[ck][:, i * HW:(i + 1) * HW])
            if i == n_ch - 1:
                pending_store.append(
                    (slice(start_ch * HW, (start_ch + n_ch) * HW),
                     stage_tiles[ck]))
            budget -= 1

    # env loads: static+dynamic host-interleaved per chunk -> ONE load DMA per
    # chunk. The HWDGE semaphore pool covers only ~10 in-flight DMAs per ring;
    # halving the DMA count doubles the ring's runway so it never drains dry
    # while a kick waits on semaphore recycling.
    w = ENV_CHUNK * HW
    env_total = EMB // ENV_CHUNK
    for c in range(env_total):
        if c == 2:
            nc.sync.dma_start(pack_t[:, HEAD_W:], t_in["small_pack"][:, HEAD_W:])
        if c == 3:
            emit_premults()
        on_gps = c % 4 == 3
        sp, op = ((env_sg_pool, env_og_pool) if on_gps
                  else (env_s_pool, env_o_pool))
        sd_tile = sp.tile([BS, 2 * w], F32, tag="env_sd")
        nc.sync.dma_start(sd_tile[:],
                          t_in["env_pack"][:, c * 2 * w:(c + 1) * 2 * w])
        o_tile = op.tile([BS, w], BF16, tag="env_o")
        eng = nc.gpsimd if on_gps else nc.vector
        eng.tensor_add(sd_tile[:, :w], sd_tile[:, :w], sd_tile[:, w:])
        eng.tensor_mul(by_ch(o_tile[:], ENV_CHUNK),
                       by_ch(sd_tile[:, :w], ENV_CHUNK),
                       bc(obs_f[:], ENV_CHUNK))
        # end-of-stream stores ride the SP ring: loads are done there, so the
        # idle ring drains the tail concurrently with ACT's backlog
        ring = nc.sync if c >= env_total - 2 else nc.scalar
        for out_cols, tile_ in pending_store:
            ring.dma_start(t_out[:, out_cols], tile_[:])
        pending_store.clear()
        pending_store.append((slice(c * w, (c + 1) * w), o_tile))
        # 2/chunk early then 1/chunk spreads the 33 small channels across the
        # whole env stream instead of draining by chunk ~19 — flattens the
        # DVE/GpSimd/ACT load so no phase of the stream is compute-co-critical
        if c >= 3:
            emit_small(2 if c <= 10 else 1)
    emit_small(len(ch_queue))
    for k, (out_cols, tile_) in enumerate(pending_store):
        (nc.sync if k % 2 == 0 else nc.scalar).dma_start(
            t_out[:, out_cols], tile_[:])
    pending_store.clear()


def build_nc():
    nc = bacc.Bacc("TRN2", target_bir_lowering=False, debug=False)
    t_in = {
        "env_pack": nc.dram_tensor(
            "env_pack", [BS, 2 * EMB * HW], F32, kind="ExternalInput"),
        "small_pack": nc.dram_tensor(
            "small_pack", [BS, PACK_W], BF16, kind="ExternalInput"),
    }
    t_out = nc.dram_tensor("out", [BS, NCH * HW], BF16, kind="ExternalOutput")
    with tile.TileContext(nc) as tc, ExitStack() as ctx:
        build_body(nc, tc, ctx, t_in, t_out)
    nc.compile()
    return nc


def make_in_maps(inputs):
    arrs = {k: np.asarray(v) for k, v in inputs.items()}
    src = {
        "obs": arrs["observability_in_memory"].reshape(B, HW),
        "rot": arrs["rotations"].reshape(B, 1).astype(np.float32),
        "obstacle": arrs["obstacle_mask"].reshape(B, HW),
        "ocur": arrs["observability_current"].reshape(B, HW),
        "leader": arrs["leader_location"].reshape(B, HW),
        "follower": arrs["follower_location"].reshape(B, HW),
        "vis": arrs["previous_visitations"].reshape(B, NROT * HW),
        "atgt": arrs["all_previous_targets"].reshape(B, NROT * HW),
        "ptgt": arrs["previous_target"].reshape(B, NROT * HW),
    }
    # interleave static/dynamic chunk-wise: [s0 d0 s1 d1 ...] so each env
    # chunk is one contiguous DMA
    es = arrs["embedded_static"].reshape(B, EMB // ENV_CHUNK, ENV_CHUNK * HW)
    ed = arrs["embedded_dynamic"].reshape(B, EMB // ENV_CHUNK, ENV_CHUNK * HW)
    env_pack = np.stack([es, ed], axis=2).reshape(B, 2 * EMB * HW)
    flat = {
        "env_pack": np.ascontiguousarray(env_pack),
        "small_pack": np.concatenate(
            [src[name].astype(NP_BF16) for name, _ in PACK_LAYOUT], axis=1),
    }
    return [
        {k: v[i * BS:(i + 1) * BS] for k, v in flat.items()}
        for i in range(N_CORES)
    ]


def gather_out(results):
    return np.concatenate(
        [np.asarray(r["out"]).astype(np.float32).reshape(BS, NCH, 25, 25)
         for r in results], axis=0)


def kernel(**inputs) -> np.ndarray:
    nc = build_nc()
    in_maps = make_in_maps(inputs)
    res = run_bass_kernel_spmd(nc, in_maps, list(range(N_CORES)))
    return gather_out(res.results)


if __name__ == "__main__":
    rng = np.random.default_rng(0)
    demo = {
        "embedded_static": rng.standard_normal((B, EMB, 25, 25), np.float32),
        "embedded_dynamic": rng.standard_normal((B, EMB, 25, 25), np.float32),
        "obstacle_mask": rng.random((B, 25, 25), dtype=np.float32),
        "observability_current": rng.random((B, 25, 25), dtype=np.float32),
        "observability_in_memory": rng.random((B, 25, 25), dtype=np.float32),
        "previous_visitations": rng.random((B, NROT, 25, 25), dtype=np.float32),
        "all_previous_targets": rng.random((B, NROT, 25, 25), dtype=np.float32),
        "previous_target": rng.random((B, NROT, 25, 25), dtype=np.float32),
        "leader_location": rng.random((B, 25, 25), dtype=np.float32),
        "follower_location": rng.random((B, 25, 25), dtype=np.float32),
        "rotations": rng.integers(0, NROT, (B,), dtype=np.int32),
    }
    out = kernel(**demo)
    print("out", out.shape, out.dtype)
